# revision 10
# baseline (speedup 1.0000x reference)
"""Trainium2 Bass kernel for the 2-layer-LSTM autoregressive sampler (AAE decoder).

Pure data parallel over batch: 8 cores x 2048 rows, weights replicated.
The categorical-sampling noise (gumbel) depends only on the fixed PRNG key, so
it is precomputed on the host exactly as jax.random.categorical would, and the
on-device sampling is argmax(logits + gumbel) with first-index tie-breaking.

Each core runs 4 sequential passes of 512 rows. Within a pass the 100-step
recurrence runs in a device-side For_i loop (2 steps per iteration,
ping-ponging h buffers so layer matmuls never race their updates).

Layout is feature-on-partition, batch-on-free:
  h/c state [128, 4, 512]  ([hidden%128, hidden//128, batch])
Embedding lookup is a one-hot matmul against XW0 = E @ W_ih0^T (E[PAD] = 0, so
a zeroed one-hot row reproduces the post-EOS padding path exactly).
"""
import sys

sys.path.insert(0, "/opt/trn_rl_repo")

import numpy as np

B, V, DEMB, H, LAT = 16384, 64, 128, 512, 128
PAD, BOS, EOS = 0, 1, 2
T = 100
NCORES = 8
BLOC = B // NCORES       # 2048 rows per core
QROWS = 512              # rows per pass
NPASS = BLOC // QROWS    # 4
P = 128
NQ = QROWS // P          # 4 m-subtiles per pass
KH = H // P              # 4 k-tiles of hidden


def legalize_waits(nc, max_waits=1):
    """walrus refuses >1 semaphore wait on many instruction types; split the
    excess onto chained same-engine NOPs placed right before the instruction
    (sequential execution on one engine makes this semantically identical)."""
    import concourse.mybir as mybir

    n_split = 0
    for fn in nc.m.functions:
        for bb in fn.blocks:
            insts = bb.instructions
            out = []
            for inst in insts:
                si = getattr(inst, "sync_info", None)
                waits = list(si.on_wait) if si is not None and si.on_wait else []
                if len(waits) > max_waits:
                    n_split += 1
                    head, tail = waits[:-max_waits], waits[-max_waits:]
                    for i in range(0, len(head), max_waits):
                        chunk = head[i:i + max_waits]
                        out.append(mybir.InstNoOp(
                            name=f"{inst.name}-waitsplit{i}",
                            engine=inst.engine,
                            ins=[],
                            outs=[],
                            sync_info=mybir.SyncInfo(on_wait=chunk, on_update=[]),
                        ))
                    si.on_wait = tail
                out.append(inst)
            if len(out) != len(insts):
                insts[:] = out
    return n_split


def build_nc(nsteps=T):
    import concourse.bass as bass
    import concourse.mybir as mybir
    from concourse.bass import ds
    from concourse.tile import TileContext

    f32 = mybir.dt.float32
    i32 = mybir.dt.int32
    AF = mybir.ActivationFunctionType
    OP = mybir.AluOpType
    AX = mybir.AxisListType

    nc = bass.Bass()

    z_p = nc.declare_dram_parameter("z", [BLOC, LAT], f32, isOutput=False)
    e_p = nc.declare_dram_parameter("E", [V, DEMB], f32, isOutput=False)
    wl2h_p = nc.declare_dram_parameter("W_l2h", [H, LAT], f32, isOutput=False)
    wih0_p = nc.declare_dram_parameter("W_ih0", [4 * H, DEMB], f32, isOutput=False)
    whh0_p = nc.declare_dram_parameter("W_hh0", [4 * H, H], f32, isOutput=False)
    wih1_p = nc.declare_dram_parameter("W_ih1", [4 * H, H], f32, isOutput=False)
    whh1_p = nc.declare_dram_parameter("W_hh1", [4 * H, H], f32, isOutput=False)
    wout_p = nc.declare_dram_parameter("W_out", [V, H], f32, isOutput=False)
    g_p = nc.declare_dram_parameter("G", [T, BLOC, V], f32, isOutput=False)
    # consts: [128, 128 ident | 64 negio | 16 b0T | 16 b1T | 4 bl2hT] = [128, 228]
    c_p = nc.declare_dram_parameter("consts", [P, 228], f32, isOutput=False)
    bos_p = nc.declare_dram_parameter("bosrow", [1, QROWS], f32, isOutput=False)
    out_p = nc.declare_dram_parameter("out", [BLOC, nsteps], i32, isOutput=True)

    with TileContext(nc) as tc:
        with (
            tc.tile_pool(name="wpool", bufs=1) as wpool,
            tc.tile_pool(name="spool", bufs=1) as spool,
            tc.tile_pool(name="wk", bufs=2) as wk,
            tc.tile_pool(name="gpsum", bufs=4, space="PSUM") as gpsum,
            tc.tile_pool(name="lpsum", bufs=2, space="PSUM") as lpsum,
            tc.tile_pool(name="tpsum", bufs=2, space="PSUM") as tpsum,
        ):
            # ---------------- consts (+ packed small weights) ----------------
            # cw: [0:228 consts][228:484 WoutT k*64+v][484:996 Wl2hT k*128+l]
            cw = wpool.tile([P, 996], f32, tag="cw")
            nc.sync.dma_start(cw[:, 0:228], c_p[:])
            ident = cw[:, 0:128]
            negio = cw[:, 128:192]             # [128, 64] rows all = 64 - v
            b0T = cw[:, 192:208]               # [128, 16]
            b1T = cw[:, 208:224]
            bl2hT = cw[:, 224:228]             # [128, 4]

            def WoutT(k):
                return cw[:, 228 + k * V:228 + (k + 1) * V]

            def Wl2hT(k):
                return cw[:, 484 + k * P:484 + (k + 1) * P]

            # ---------------- weights (one-time transposes) ----------------
            WhhT0 = wpool.tile([P, KH, 4 * H], f32, tag="WhhT0")
            WihT1 = wpool.tile([P, KH, 4 * H], f32, tag="WihT1")
            WhhT1 = wpool.tile([P, KH, 4 * H], f32, tag="WhhT1")
            XW0 = wpool.tile([V, 4 * H], f32, tag="XW0")

            with tc.tile_pool(name="pre", bufs=1) as pre:
                for (src, dstT) in ((whh0_p, WhhT0), (wih1_p, WihT1), (whh1_p, WhhT1)):
                    for jt in range(16):
                        nat = pre.tile([P, H], f32, tag="nat")
                        nc.sync.dma_start(nat[:], src[jt * P:(jt + 1) * P, :])
                        for k in range(KH):
                            tp = tpsum.tile([P, P], f32, tag="tp")
                            nc.tensor.transpose(tp[:], nat[:, k * P:(k + 1) * P], ident)
                            nc.vector.tensor_copy(dstT[:, k, jt * P:(jt + 1) * P], tp[:])
                # W_out [64, 512] -> WoutT [128(h%128), 4, 64]
                wo = pre.tile([V, H], f32, tag="wo")
                nc.sync.dma_start(wo[:], wout_p[:])
                for k in range(KH):
                    tp = tpsum.tile([P, P], f32, tag="tp")
                    nc.tensor.transpose(tp[:], wo[:, k * P:(k + 1) * P], ident[:V, :])
                    nc.vector.tensor_copy(WoutT(k), tp[:, :V])
                # W_l2h [512, 128] -> Wl2hT [128(l), 4, 128(h)]
                wl = pre.tile([P, KH, LAT], f32, tag="wl")
                nc.sync.dma_start(wl[:], wl2h_p[:].rearrange("(k p) l -> p k l", p=P))
                for k in range(KH):
                    tp = tpsum.tile([P, P], f32, tag="tp")
                    nc.tensor.transpose(tp[:], wl[:, k, :], ident)
                    nc.vector.tensor_copy(Wl2hT(k), tp[:])
                # XW0 = E @ W_ih0^T  [64, 2048]
                en = pre.tile([V, DEMB], f32, tag="wo")
                nc.sync.dma_start(en[:], e_p[:])
                ET = pre.tile([DEMB, V], f32, tag="ET")
                tp = tpsum.tile([P, P], f32, tag="tp")
                nc.tensor.transpose(tp[:], en[:], ident[:V, :])
                nc.vector.tensor_copy(ET[:], tp[:, :V])
                w0T = pre.tile([DEMB, 4 * H], f32, tag="w0T")
                for jt in range(16):
                    nat = pre.tile([P, DEMB], f32, tag="nat")
                    nc.sync.dma_start(nat[:], wih0_p[jt * P:(jt + 1) * P, :])
                    tp = tpsum.tile([P, P], f32, tag="tp")
                    nc.tensor.transpose(tp[:], nat[:], ident)
                    nc.vector.tensor_copy(w0T[:, jt * P:(jt + 1) * P], tp[:])
                for n in range(4):
                    xp = gpsum.tile([P, H], f32, tag="gp")
                    nc.tensor.matmul(xp[:V, :], ET[:], w0T[:, n * H:(n + 1) * H],
                                     start=True, stop=True)
                    nc.vector.tensor_copy(XW0[:, n * H:(n + 1) * H], xp[:V, :])

            # ---------------- persistent state ----------------
            hA0 = spool.tile([P, KH, QROWS], f32, tag="hA0")
            hB0 = spool.tile([P, KH, QROWS], f32, tag="hB0")
            hA1 = spool.tile([P, KH, QROWS], f32, tag="hA1")
            hB1 = spool.tile([P, KH, QROWS], f32, tag="hB1")
            c0 = spool.tile([P, KH, QROWS], f32, tag="c0")
            c1 = spool.tile([P, KH, QROWS], f32, tag="c1")
            ohP = spool.tile([V, 2, QROWS], f32, tag="ohP")
            ohA = ohP[:, 0, :]
            ohB = ohP[:, 1, :]
            # small scratch: per-m stride 384: sc@0 eq@64 msk@128 oh@192 ohm@256
            # mx@320 rr@321 idx@322 eosn@323 ie@324 alive@325 ; is_end@1536+m
            smx = spool.tile([P, 1548], f32, tag="smx")
            outb = spool.tile([P, NQ, nsteps // 2, 2], i32, tag="outb")

            # [q, u, p, s, m, v]
            g_view = g_p[:].rearrange("(s u) (q m p) v -> q u p s m v",
                                      u=2, q=NPASS, p=P)

            def lstm_layer(biasT, WT_list, oh_rhs, cstate, h_out):
                nmm = sum(len(hl) for _, hl in WT_list)
                for q in range(KH):
                    ag = [None] * 4
                    for g in range(4):
                        j = g * KH + q
                        ps = gpsum.tile([P, QROWS], f32, tag="gp")
                        first = True
                        if oh_rhs is not None:
                            nc.tensor.matmul(ps[:], XW0[:, j * P:(j + 1) * P], oh_rhs,
                                             start=True, stop=False)
                            first = False
                        mi = 0
                        for WT, h_list in WT_list:
                            for k, hk in h_list:
                                mi += 1
                                nc.tensor.matmul(
                                    ps[:], WT[:, k, j * P:(j + 1) * P], hk,
                                    start=first, stop=(mi == nmm))
                                first = False
                        agt = wk.tile([P, QROWS], f32, tag=f"ag{g}", bufs=1)
                        fn = AF.Tanh if g == 2 else AF.Sigmoid
                        nc.scalar.activation(agt[:], ps[:], fn, bias=biasT[:, j:j + 1])
                        ag[g] = agt
                    # c = sig(f)*c + sig(i)*tanh(g); h = sig(o)*tanh(c)
                    t1 = wk.tile([P, QROWS], f32, tag="t1")
                    nc.vector.tensor_tensor(t1[:], ag[1][:], cstate[:, q, :], OP.mult)
                    t2 = wk.tile([P, QROWS], f32, tag="t2")
                    nc.vector.tensor_tensor(t2[:], ag[0][:], ag[2][:], OP.mult)
                    nc.vector.tensor_tensor(cstate[:, q, :], t1[:], t2[:], OP.add)
                    tch = wk.tile([P, QROWS], f32, tag="t1", name="tch")
                    nc.scalar.activation(tch[:], cstate[:, q, :], AF.Tanh)
                    nc.vector.tensor_tensor(h_out[:, q, :], ag[3][:], tch[:], OP.mult)

            def step(s, sub, h0r, h0w, h1r, h1w, ohr, ohw, gum):
                # layer 0: gates = onehot @ XW0 + h0 @ Whh0^T + b0
                lstm_layer(b0T,
                           [(WhhT0, [(k, h0r[:, k, :]) for k in range(KH)])],
                           ohr[:], c0, h0w)
                # layer 1: gates = h0_new @ Wih1^T + h1 @ Whh1^T + b1
                lstm_layer(b1T,
                           [(WihT1, [(k, h0w[:, k, :]) for k in range(KH)]),
                            (WhhT1, [(k, h1r[:, k, :]) for k in range(KH)])],
                           None, c1, h1w)
                # logits + sampling per 128-row subtile
                for m in range(NQ):
                    pl = lpsum.tile([P, V], f32, tag="lp")
                    for k in range(KH):
                        nc.tensor.matmul(pl[:], h1w[:, k, m * P:(m + 1) * P],
                                         WoutT(k),
                                         start=(k == 0), stop=(k == KH - 1))
                    o = m * 384
                    sc = smx[:, o + 0:o + 64]
                    eq = smx[:, o + 64:o + 128]
                    msk = smx[:, o + 128:o + 192]
                    oh = smx[:, o + 192:o + 256]
                    ohm = smx[:, o + 256:o + 320]
                    mx = smx[:, o + 320:o + 321]
                    rr = smx[:, o + 321:o + 322]
                    idx = smx[:, o + 322:o + 323]
                    eosn = smx[:, o + 323:o + 324]
                    ie = smx[:, o + 324:o + 325]
                    alive = smx[:, o + 325:o + 326]
                    iend = smx[:, 1536 + m:1537 + m]
                    nc.vector.tensor_tensor(sc, pl[:], gum[:, sub, m, :], OP.add)
                    nc.vector.reduce_max(mx, sc, axis=AX.X)
                    nc.vector.tensor_scalar(eq, sc, mx, None, OP.is_equal)
                    nc.vector.tensor_tensor(msk, eq, negio, OP.mult)
                    nc.vector.reduce_max(rr, msk, axis=AX.X)
                    # idx = 64 - rr ; first-max index, matching jnp.argmax
                    nc.vector.tensor_scalar(idx, rr, -1.0, float(V),
                                            OP.mult, OP.add)
                    nc.vector.tensor_scalar(eosn, idx, float(EOS), None,
                                            OP.is_equal)
                    nc.vector.tensor_tensor(ie, iend, eosn, OP.max)
                    nc.vector.tensor_copy(iend, ie)
                    nc.vector.tensor_scalar(alive, ie, -1.0, 1.0,
                                            OP.mult, OP.add)
                    tok = smx[:, 1540 + m:1541 + m]
                    nc.vector.tensor_tensor(tok, idx, alive, OP.mult)
                    nc.vector.tensor_scalar(oh, negio, rr, None, OP.is_equal)
                    nc.vector.tensor_scalar(ohm, oh, alive, None, OP.mult)
                    tp = tpsum.tile([P, P], f32, tag="tp")
                    nc.tensor.transpose(tp[:V, :], ohm, ident)
                    nc.scalar.copy(ohw[:, m * P:(m + 1) * P], tp[:V, :])
                # tokens of all 4 subtiles -> output column t = 2s + sub
                nc.vector.tensor_copy(outb[:, :, ds(s, 1), sub:sub + 1],
                                      smx[:, 1540:1544])

            for q_pass in range(NPASS):
                # ---- per-pass init ----
                for st in (hA0, hB0, hA1, hB1):
                    nc.vector.memset(st[:], 0.0)
                nc.vector.memset(smx[:, 1536:1540], 0.0)
                nc.vector.memset(ohA[:], 0.0)
                nc.sync.dma_start(ohA[BOS:BOS + 1, :], bos_p[:])
                zT = wk.tile([P, QROWS], f32, tag="ag0", bufs=1, name="zT")
                for m in range(NQ):
                    znat = wk.tile([P, LAT], f32, tag="ag1", bufs=1, name="znat")
                    r0 = q_pass * QROWS + m * P
                    nc.sync.dma_start(znat[:], z_p[r0:r0 + P, :])
                    tp = tpsum.tile([P, P], f32, tag="tp")
                    nc.tensor.transpose(tp[:], znat[:], ident)
                    nc.vector.tensor_copy(zT[:, m * P:(m + 1) * P], tp[:])
                for k in range(KH):
                    cp = gpsum.tile([P, QROWS], f32, tag="gp")
                    nc.tensor.matmul(cp[:], Wl2hT(k), zT[:],
                                     start=True, stop=True)
                    nc.vector.tensor_scalar(c0[:, k, :], cp[:],
                                            bl2hT[:, k:k + 1], None, OP.add)
                    nc.vector.tensor_copy(c1[:, k, :], c0[:, k, :])

                with tc.For_i(0, nsteps // 2) as s:
                    gum = wk.tile([P, 2, NQ, V], f32, tag="gum")
                    for u in range(2):
                        nc.sync.dma_start(gum[:, u:u + 1, :, :],
                                          g_view[q_pass, u, :, ds(s, 1), :, :])
                    step(s, 0, hA0, hB0, hA1, hB1, ohA, ohB, gum)
                    step(s, 1, hB0, hA0, hB1, hA1, ohB, ohA, gum)

                ov = out_p[q_pass * QROWS:(q_pass + 1) * QROWS, :]
                nc.sync.dma_start(
                    ov.rearrange("(m p) (s u) -> p m s u", p=P, u=2), outb[:])

    legalize_waits(nc)
    return nc


def _host_consts(b_ih0, b_hh0, b_ih1, b_hh1, b_l2h):
    consts = np.zeros((P, 228), np.float32)
    consts[:, 0:128] = np.eye(P, dtype=np.float32)
    consts[:, 128:192] = (V - np.arange(V, dtype=np.float32))[None, :]
    consts[:, 192:208] = (b_ih0 + b_hh0).astype(np.float32).reshape(16, P).T
    consts[:, 208:224] = (b_ih1 + b_hh1).astype(np.float32).reshape(16, P).T
    consts[:, 224:228] = b_l2h.astype(np.float32).reshape(KH, P).T
    return consts


def _host_gumbel(b_out):
    """Exactly the noise jax.random.categorical(keys[t], logits) adds."""
    import jax

    cpu = jax.devices("cpu")[0]
    with jax.default_device(cpu):
        keys = jax.random.split(jax.random.key(42), T)

        def one(k):
            return jax.random.gumbel(k, (B, V), "float32")

        one_j = jax.jit(one)
        # NOTE: vmap over keys generates different bits than per-key calls;
        # the reference samples per key, so loop.
        g = np.stack([np.asarray(one_j(keys[t])) for t in range(T)])
    g = g + b_out.astype(np.float32)[None, None, :]
    return g


def kernel(z, E, W_l2h, b_l2h, W_ih0, W_hh0, b_ih0, b_hh0,
           W_ih1, W_hh1, b_ih1, b_hh1, W_out, b_out, max_len):
    from concourse.bass_utils import run_bass_kernel_spmd

    assert int(max_len) == T
    consts = _host_consts(np.asarray(b_ih0), np.asarray(b_hh0),
                          np.asarray(b_ih1), np.asarray(b_hh1),
                          np.asarray(b_l2h))
    G = _host_gumbel(np.asarray(b_out))

    nc = build_nc()
    f32 = np.float32
    shared = {
        "E": np.ascontiguousarray(np.asarray(E, f32)),
        "W_l2h": np.ascontiguousarray(np.asarray(W_l2h, f32)),
        "W_ih0": np.ascontiguousarray(np.asarray(W_ih0, f32)),
        "W_hh0": np.ascontiguousarray(np.asarray(W_hh0, f32)),
        "W_ih1": np.ascontiguousarray(np.asarray(W_ih1, f32)),
        "W_hh1": np.ascontiguousarray(np.asarray(W_hh1, f32)),
        "W_out": np.ascontiguousarray(np.asarray(W_out, f32)),
        "consts": consts,
        "bosrow": np.ones((1, QROWS), np.float32),
    }
    z = np.asarray(z, f32)
    in_maps = []
    for i in range(NCORES):
        rows = slice(i * BLOC, (i + 1) * BLOC)
        m = dict(shared)
        m["z"] = np.ascontiguousarray(z[rows])
        m["G"] = np.ascontiguousarray(G[:, rows, :])
        in_maps.append(m)

    res = run_bass_kernel_spmd(nc, in_maps, list(range(NCORES)))
    out = np.concatenate([res.results[i]["out"] for i in range(NCORES)], axis=0)
    return out.astype(np.int32)


if __name__ == "__main__":
    import reference as R

    inputs = R.setup_inputs()
    out = kernel(**{k: np.asarray(v) if hasattr(v, "shape") else v
                    for k, v in inputs.items()})
    print(out.shape, out.dtype)
    print(out[:2, :20])


# revision 13
# speedup vs baseline: 12.0316x; 12.0316x over previous
"""Trainium2 Bass kernel for the 2-layer-LSTM autoregressive sampler (AAE decoder).

Same algorithm as kernel.py (exact-sampling via host-precomputed gumbel,
one-hot embedding matmul, batch data-parallel over 8 cores x 4 passes of 512
rows), but the recurrent gate matmuls run as fp16 DUAL-PLANE products:
  W = Whi + Wlo/2048,  h = hhi + hlo/2048   (fp16 planes, lo pre-scaled)
  W@h ~= [Whi@hhi] + ([Whi@hlo + Wlo@hhi]) / 2048      (lo*lo term dropped)
Three fp16 matmuls (1 cyc/row) replace one fp32 matmul (4 cyc/row) at a
~2^-22 relative residual - fp32-grade. The logits matmul stays genuine fp32
(h1 kept in f32 for it), so the sampled scores keep full precision.
All weight planes are prepared host-side in the transposed SBUF layout.
"""
import sys

sys.path.insert(0, "/opt/trn_rl_repo")

import numpy as np

B, V, DEMB, H, LAT = 16384, 64, 128, 512, 128
PAD, BOS, EOS = 0, 1, 2
T = 100
NCORES = 8
BLOC = B // NCORES       # 2048 rows per core
QROWS = 512              # rows per pass
NPASS = BLOC // QROWS    # 4
P = 128
NQ = QROWS // P          # 4 m-subtiles per pass
KH = H // P              # 4 k-tiles of hidden
LSCALE = 2048.0          # lo-plane pre-scale (2^11)
F16_MIN_NORMAL = 6.104e-05


def legalize_waits(nc, max_waits=1):
    """walrus refuses >1 semaphore wait on many instruction types; split the
    excess onto chained same-engine NOPs placed right before the instruction
    (sequential execution on one engine makes this semantically identical)."""
    import concourse.mybir as mybir

    n_split = 0
    for fn in nc.m.functions:
        for bb in fn.blocks:
            insts = bb.instructions
            out = []
            for inst in insts:
                si = getattr(inst, "sync_info", None)
                waits = list(si.on_wait) if si is not None and si.on_wait else []
                if len(waits) > max_waits:
                    n_split += 1
                    head, tail = waits[:-max_waits], waits[-max_waits:]
                    for i in range(0, len(head), max_waits):
                        chunk = head[i:i + max_waits]
                        out.append(mybir.InstNoOp(
                            name=f"{inst.name}-waitsplit{i}",
                            engine=inst.engine,
                            ins=[],
                            outs=[],
                            sync_info=mybir.SyncInfo(on_wait=chunk, on_update=[]),
                        ))
                    si.on_wait = tail
                out.append(inst)
            if len(out) != len(insts):
                insts[:] = out
    return n_split


def build_nc(nsteps=T, reps=1):
    import contextlib
    import concourse.bass as bass
    import concourse.mybir as mybir
    from concourse.bass import ds
    from concourse.tile import TileContext

    f32 = mybir.dt.float32
    f16 = mybir.dt.float16
    i32 = mybir.dt.int32
    AF = mybir.ActivationFunctionType
    OP = mybir.AluOpType
    AX = mybir.AxisListType

    nc = bass.Bass()

    z_p = nc.declare_dram_parameter("z", [BLOC, LAT], f32, isOutput=False)
    # fp16 weight planes, already in [p, k, j] transposed layout
    w0h_p = nc.declare_dram_parameter("w0h", [P, KH, 4 * H], f16, isOutput=False)
    w0l_p = nc.declare_dram_parameter("w0l", [P, KH, 4 * H], f16, isOutput=False)
    w1ih_h_p = nc.declare_dram_parameter("w1ihh", [P, KH, 4 * H], f16, isOutput=False)
    w1ih_l_p = nc.declare_dram_parameter("w1ihl", [P, KH, 4 * H], f16, isOutput=False)
    w1hh_h_p = nc.declare_dram_parameter("w1hhh", [P, KH, 4 * H], f16, isOutput=False)
    w1hh_l_p = nc.declare_dram_parameter("w1hhl", [P, KH, 4 * H], f16, isOutput=False)
    xw0h_p = nc.declare_dram_parameter("xw0h", [V, 4 * H], f16, isOutput=False)
    xw0l_p = nc.declare_dram_parameter("xw0l", [V, 4 * H], f16, isOutput=False)
    g_p = nc.declare_dram_parameter("G", [T, BLOC, V], f32, isOutput=False)
    # consts: [0:128 ident | 128:192 negio | 192:208 b0T | 208:224 b1T |
    #          224:228 bl2hT | 228:484 WoutT (k*64+v) | 484:996 Wl2hT (k*128+l)]
    c_p = nc.declare_dram_parameter("consts", [P, 996], f32, isOutput=False)
    bos_p = nc.declare_dram_parameter("bosrow", [1, QROWS], f16, isOutput=False)
    out_p = nc.declare_dram_parameter("out", [BLOC, nsteps], i32, isOutput=True)

    with TileContext(nc) as tc:
        with (
            tc.tile_pool(name="wpool", bufs=1) as wpool,
            tc.tile_pool(name="spool", bufs=1) as spool,
            tc.tile_pool(name="wk", bufs=2) as wk,
            tc.tile_pool(name="gpsum", bufs=3, space="PSUM") as gpsum,
            tc.tile_pool(name="lpsum", bufs=1, space="PSUM") as lpsum,
            tc.tile_pool(name="tpsum", bufs=1, space="PSUM") as tpsum,
        ):
            cw = wpool.tile([P, 996], f32, tag="cw")
            nc.sync.dma_start(cw[:], c_p[:])
            ident = cw[:, 0:128]
            negio = cw[:, 128:192]
            b0T = cw[:, 192:208]
            b1T = cw[:, 208:224]
            bl2hT = cw[:, 224:228]

            def WoutT(k):
                return cw[:, 228 + k * V:228 + (k + 1) * V]

            def Wl2hT(k):
                return cw[:, 484 + k * P:484 + (k + 1) * P]

            w0h = wpool.tile([P, KH, 4 * H], f16, tag="w0h")
            w0l = wpool.tile([P, KH, 4 * H], f16, tag="w0l")
            w1ihh = wpool.tile([P, KH, 4 * H], f16, tag="w1ihh")
            w1ihl = wpool.tile([P, KH, 4 * H], f16, tag="w1ihl")
            w1hhh = wpool.tile([P, KH, 4 * H], f16, tag="w1hhh")
            w1hhl = wpool.tile([P, KH, 4 * H], f16, tag="w1hhl")
            xw0h = wpool.tile([V, 4 * H], f16, tag="xw0h")
            xw0l = wpool.tile([V, 4 * H], f16, tag="xw0l")
            for t_, p_ in ((w0h, w0h_p), (w0l, w0l_p), (w1ihh, w1ih_h_p),
                           (w1ihl, w1ih_l_p), (w1hhh, w1hh_h_p),
                           (w1hhl, w1hh_l_p), (xw0h, xw0h_p), (xw0l, xw0l_p)):
                nc.sync.dma_start(t_[:], p_[:])

            # ---------------- persistent state ----------------
            def st(name, dtype=f16):
                return spool.tile([P, KH, QROWS], dtype, tag=name, name=name)

            hA0h, hA0l = st("hA0h"), st("hA0l")
            hB0h, hB0l = st("hB0h"), st("hB0l")
            hA1h, hA1l = st("hA1h"), st("hA1l")
            hB1h, hB1l = st("hB1h"), st("hB1l")
            c0 = st("c0", f32)
            c1 = st("c1", f32)
            h1f32 = st("h1f32", f32)
            ohP = spool.tile([V, 2, QROWS], f16, tag="ohP")
            ohA = ohP[:, 0, :]
            ohB = ohP[:, 1, :]
            smx = spool.tile([P, 1548], f32, tag="smx")
            outb = spool.tile([P, NQ, nsteps // 2, 2], i32, tag="outb")

            g_view = g_p[:].rearrange("(s u) (q m p) v -> q u p s m v",
                                      u=2, q=NPASS, p=P)

            def gates_layer(biasT, hi_terms, cross_terms, oh_pair, cstate,
                            store_h):
                """hi_terms: [(Wplane, hplane)] -> ps1; cross_terms -> ps2;
                oh_pair: (xh, xl, ohr) or None. comb = ps1 + ps2/LSCALE."""
                for q in range(KH):
                    ag = [None] * 4
                    for g in range(4):
                        j = g * KH + q
                        jsl = slice(j * P, (j + 1) * P)
                        ps1 = gpsum.tile([P, QROWS], f32, tag="gp1", name="ps1")
                        ps2 = gpsum.tile([P, QROWS], f32, tag="gp2", name="ps2")
                        n1 = len(hi_terms) * KH + (1 if oh_pair else 0)
                        mi = 0
                        if oh_pair:
                            mi += 1
                            nc.tensor.matmul(ps1[:], oh_pair[0][:, jsl],
                                             oh_pair[2], start=True,
                                             stop=(mi == n1))
                        for (W, hp) in hi_terms:
                            for k in range(KH):
                                mi += 1
                                nc.tensor.matmul(
                                    ps1[:], W[:, k, jsl], hp[:, k, :],
                                    start=(mi == 1), stop=(mi == n1))
                        n2 = len(cross_terms) * KH + (1 if oh_pair else 0)
                        mi = 0
                        if oh_pair:
                            mi += 1
                            nc.tensor.matmul(ps2[:], oh_pair[1][:, jsl],
                                             oh_pair[2], start=True,
                                             stop=(mi == n2))
                        for (W, hp) in cross_terms:
                            for k in range(KH):
                                mi += 1
                                nc.tensor.matmul(
                                    ps2[:], W[:, k, jsl], hp[:, k, :],
                                    start=(mi == 1), stop=(mi == n2))
                        comb = wk.tile([P, QROWS], f32, tag="comb")
                        nc.vector.tensor_scalar(comb[:], ps2[:], 1.0 / LSCALE,
                                                None, OP.mult)
                        nc.vector.tensor_tensor(comb[:], comb[:], ps1[:],
                                                OP.add)
                        agt = wk.tile([P, QROWS], f32, tag=f"ag{g}", bufs=1,
                                      name=f"ag{g}")
                        fn = AF.Tanh if g == 2 else AF.Sigmoid
                        nc.scalar.activation(agt[:], comb[:], fn,
                                             bias=biasT[:, j:j + 1])
                        ag[g] = agt
                    t1 = wk.tile([P, QROWS], f32, tag="t1")
                    nc.vector.tensor_tensor(t1[:], ag[1][:], cstate[:, q, :],
                                            OP.mult)
                    t2 = wk.tile([P, QROWS], f32, tag="t2")
                    nc.vector.tensor_tensor(t2[:], ag[0][:], ag[2][:], OP.mult)
                    nc.vector.tensor_tensor(cstate[:, q, :], t1[:], t2[:],
                                            OP.add)
                    tch = wk.tile([P, QROWS], f32, tag="t1", name="tch")
                    nc.scalar.activation(tch[:], cstate[:, q, :], AF.Tanh)
                    store_h(q, ag[3], tch)

            def split_h(q, hnew_f32, hi_dst, lo_dst):
                """hi = f16(h); lo = f16((h - hi) * 2048)."""
                nc.vector.tensor_copy(hi_dst[:, q, :], hnew_f32)
                d = wk.tile([P, QROWS], f32, tag="t2", name="dsplit")
                nc.vector.tensor_tensor(d[:], hnew_f32, hi_dst[:, q, :],
                                        OP.subtract)
                nc.vector.tensor_scalar(lo_dst[:, q, :], d[:], LSCALE, None,
                                        OP.mult)

            def step(s, sub, h0rh, h0rl, h0wh, h0wl, h1rh, h1rl, h1wh, h1wl,
                     ohr, ohw, gum):
                # ---- layer 0 ----
                def store0(q, ago, tch):
                    hn = wk.tile([P, QROWS], f32, tag="t2", name="hnew0")
                    nc.vector.tensor_tensor(hn[:], ago[:], tch[:], OP.mult)
                    split_h(q, hn[:], h0wh, h0wl)

                gates_layer(b0T, [(w0h, h0rh)],
                            [(w0l, h0rh), (w0h, h0rl)],
                            (xw0h, xw0l, ohr[:]), c0, store0)

                # ---- layer 1 ----
                def store1(q, ago, tch):
                    nc.vector.tensor_tensor(h1f32[:, q, :], ago[:], tch[:],
                                            OP.mult)
                    split_h(q, h1f32[:, q, :], h1wh, h1wl)

                gates_layer(b1T, [(w1ihh, h0wh), (w1hhh, h1rh)],
                            [(w1ihl, h0wh), (w1ihh, h0wl),
                             (w1hhl, h1rh), (w1hhh, h1rl)],
                            None, c1, store1)

                # ---- logits + sampling (full fp32) ----
                for m in range(NQ):
                    pl = lpsum.tile([P, V], f32, tag="lp")
                    for k in range(KH):
                        nc.tensor.matmul(pl[:], h1f32[:, k, m * P:(m + 1) * P],
                                         WoutT(k),
                                         start=(k == 0), stop=(k == KH - 1))
                    o = m * 384
                    sc = smx[:, o + 0:o + 64]
                    eq = smx[:, o + 64:o + 128]
                    msk = smx[:, o + 128:o + 192]
                    oh = smx[:, o + 192:o + 256]
                    ohm = smx[:, o + 256:o + 320]
                    mx = smx[:, o + 320:o + 321]
                    rr = smx[:, o + 321:o + 322]
                    idx = smx[:, o + 322:o + 323]
                    eosn = smx[:, o + 323:o + 324]
                    ie = smx[:, o + 324:o + 325]
                    alive = smx[:, o + 325:o + 326]
                    iend = smx[:, 1536 + m:1537 + m]
                    nc.vector.tensor_tensor(sc, pl[:], gum[:, sub, m, :],
                                            OP.add)
                    nc.vector.reduce_max(mx, sc, axis=AX.X)
                    nc.vector.tensor_scalar(eq, sc, mx, None, OP.is_equal)
                    nc.vector.tensor_tensor(msk, eq, negio, OP.mult)
                    nc.vector.reduce_max(rr, msk, axis=AX.X)
                    nc.vector.tensor_scalar(idx, rr, -1.0, float(V),
                                            OP.mult, OP.add)
                    nc.vector.tensor_scalar(eosn, idx, float(EOS), None,
                                            OP.is_equal)
                    nc.vector.tensor_tensor(ie, iend, eosn, OP.max)
                    nc.vector.tensor_copy(iend, ie)
                    nc.vector.tensor_scalar(alive, ie, -1.0, 1.0,
                                            OP.mult, OP.add)
                    tok = smx[:, 1540 + m:1541 + m]
                    nc.vector.tensor_tensor(tok, idx, alive, OP.mult)
                    nc.vector.tensor_scalar(oh, negio, rr, None, OP.is_equal)
                    nc.vector.tensor_scalar(ohm, oh, alive, None, OP.mult)
                    tp = tpsum.tile([P, P], f32, tag="tp")
                    nc.tensor.transpose(tp[:V, :], ohm, ident)
                    nc.scalar.copy(ohw[:, m * P:(m + 1) * P], tp[:V, :])
                nc.vector.tensor_copy(outb[:, :, ds(s, 1), sub:sub + 1],
                                      smx[:, 1540:1544])

            rep_ctx = tc.For_i(0, reps) if reps > 1 else contextlib.nullcontext()
            with rep_ctx:
              for q_pass in range(NPASS):
                for s_ in (hA0h, hA0l, hB0h, hB0l, hA1h, hA1l, hB1h, hB1l,
                           h1f32):
                    nc.vector.memset(s_[:], 0.0)
                nc.vector.memset(smx[:, 1536:1540], 0.0)
                nc.vector.memset(ohP[:], 0.0)
                nc.sync.dma_start(ohA[BOS:BOS + 1, :], bos_p[:])
                zT = wk.tile([P, QROWS], f32, tag="ag0", bufs=1, name="zT")
                for m in range(NQ):
                    znat = wk.tile([P, LAT], f32, tag="ag1", bufs=1,
                                   name="znat")
                    r0 = q_pass * QROWS + m * P
                    nc.sync.dma_start(znat[:], z_p[r0:r0 + P, :])
                    tp = tpsum.tile([P, P], f32, tag="tp")
                    nc.tensor.transpose(tp[:], znat[:], ident)
                    nc.vector.tensor_copy(zT[:, m * P:(m + 1) * P], tp[:])
                for k in range(KH):
                    cp = gpsum.tile([P, QROWS], f32, tag="gp1", name="cinit")
                    nc.tensor.matmul(cp[:], Wl2hT(k), zT[:],
                                     start=True, stop=True)
                    nc.vector.tensor_scalar(c0[:, k, :], cp[:],
                                            bl2hT[:, k:k + 1], None, OP.add)
                    nc.vector.tensor_copy(c1[:, k, :], c0[:, k, :])

                with tc.For_i(0, nsteps // 2) as s:
                    gum = wk.tile([P, 2, NQ, V], f32, tag="gum", bufs=1,
                                  name="gum")
                    for u in range(2):
                        nc.sync.dma_start(gum[:, u:u + 1, :, :],
                                          g_view[q_pass, u, :, ds(s, 1), :, :])
                    step(s, 0, hA0h, hA0l, hB0h, hB0l, hA1h, hA1l,
                         hB1h, hB1l, ohA, ohB, gum)
                    step(s, 1, hB0h, hB0l, hA0h, hA0l, hB1h, hB1l,
                         hA1h, hA1l, ohB, ohA, gum)

                ov = out_p[q_pass * QROWS:(q_pass + 1) * QROWS, :]
                nc.sync.dma_start(
                    ov.rearrange("(m p) (s u) -> p m s u", p=P, u=2), outb[:])

    legalize_waits(nc)
    return nc


def _split16(W):
    """fp16 dual planes with subnormal-flushed hi."""
    hi = W.astype(np.float16)
    hi[np.abs(hi.astype(np.float32)) < F16_MIN_NORMAL] = 0
    lo = ((W - hi.astype(np.float32)) * LSCALE).astype(np.float16)
    return hi, lo


def _tlayout(W):
    """[4H, K] weight -> [p, k, j] transposed plane layout ([128, KH, 4H])."""
    K = W.shape[1]
    return np.ascontiguousarray(
        W.T.reshape(K // P, P, 4 * H).transpose(1, 0, 2))


def _host_consts(b_ih0, b_hh0, b_ih1, b_hh1, b_l2h, W_out, W_l2h):
    consts = np.zeros((P, 996), np.float32)
    consts[:, 0:128] = np.eye(P, dtype=np.float32)
    consts[:, 128:192] = (V - np.arange(V, dtype=np.float32))[None, :]
    consts[:, 192:208] = (b_ih0 + b_hh0).astype(np.float32).reshape(16, P).T
    consts[:, 208:224] = (b_ih1 + b_hh1).astype(np.float32).reshape(16, P).T
    consts[:, 224:228] = b_l2h.astype(np.float32).reshape(KH, P).T
    # WoutT[p, k, v] = W_out[v, k*128+p]
    wt = W_out.astype(np.float32).T.reshape(KH, P, V).transpose(1, 0, 2)
    consts[:, 228:484] = wt.reshape(P, KH * V)
    # Wl2hT[p(l), k, m] = W_l2h[k*128+m, p(l)]
    wl = W_l2h.astype(np.float32).reshape(KH, P, LAT).transpose(2, 0, 1)
    consts[:, 484:996] = wl.reshape(P, KH * P)
    return consts


def _host_gumbel(b_out):
    """Exactly the noise jax.random.categorical(keys[t], logits) adds."""
    import jax

    cpu = jax.devices("cpu")[0]
    with jax.default_device(cpu):
        keys = jax.random.split(jax.random.key(42), T)

        def one(k):
            return jax.random.gumbel(k, (B, V), "float32")

        one_j = jax.jit(one)
        # NOTE: vmap over keys generates different bits than per-key calls;
        # the reference samples per key, so loop.
        g = np.stack([np.asarray(one_j(keys[t])) for t in range(T)])
    g = g + b_out.astype(np.float32)[None, None, :]
    return g


def prepare_in_maps(inputs):
    """Build the per-core SPMD input maps from the full problem inputs."""
    f32 = np.float32
    E = np.asarray(inputs["E"], f32)
    W_ih0 = np.asarray(inputs["W_ih0"], f32)
    consts = _host_consts(np.asarray(inputs["b_ih0"]), np.asarray(inputs["b_hh0"]),
                          np.asarray(inputs["b_ih1"]), np.asarray(inputs["b_hh1"]),
                          np.asarray(inputs["b_l2h"]),
                          np.asarray(inputs["W_out"], f32),
                          np.asarray(inputs["W_l2h"], f32))
    G = _host_gumbel(np.asarray(inputs["b_out"]))

    XW0 = (E @ W_ih0.T).astype(f32)               # [64, 2048]
    xw0h, xw0l = _split16(XW0)
    w0h, w0l = _split16(_tlayout(np.asarray(inputs["W_hh0"], f32)))
    w1ihh, w1ihl = _split16(_tlayout(np.asarray(inputs["W_ih1"], f32)))
    w1hhh, w1hhl = _split16(_tlayout(np.asarray(inputs["W_hh1"], f32)))

    shared = {
        "w0h": w0h, "w0l": w0l, "w1ihh": w1ihh, "w1ihl": w1ihl,
        "w1hhh": w1hhh, "w1hhl": w1hhl,
        "xw0h": np.ascontiguousarray(xw0h),
        "xw0l": np.ascontiguousarray(xw0l),
        "consts": consts,
        "bosrow": np.ones((1, QROWS), np.float16),
    }
    z = np.asarray(inputs["z"], f32)
    in_maps = []
    for i in range(NCORES):
        rows = slice(i * BLOC, (i + 1) * BLOC)
        m = dict(shared)
        m["z"] = np.ascontiguousarray(z[rows])
        m["G"] = np.ascontiguousarray(G[:, rows, :])
        in_maps.append(m)
    return in_maps


def kernel(z, E, W_l2h, b_l2h, W_ih0, W_hh0, b_ih0, b_hh0,
           W_ih1, W_hh1, b_ih1, b_hh1, W_out, b_out, max_len):
    from concourse.bass_utils import run_bass_kernel_spmd

    assert int(max_len) == T
    in_maps = prepare_in_maps(dict(
        z=z, E=E, W_l2h=W_l2h, b_l2h=b_l2h, W_ih0=W_ih0, W_hh0=W_hh0,
        b_ih0=b_ih0, b_hh0=b_hh0, W_ih1=W_ih1, W_hh1=W_hh1, b_ih1=b_ih1,
        b_hh1=b_hh1, W_out=W_out, b_out=b_out))
    nc = build_nc()
    res = run_bass_kernel_spmd(nc, in_maps, list(range(NCORES)))
    out = np.concatenate([res.results[i]["out"] for i in range(NCORES)], axis=0)
    return out.astype(np.int32)


# revision 14
# speedup vs baseline: 13.8646x; 1.1523x over previous
"""Trainium2 Bass kernel for the 2-layer-LSTM autoregressive sampler (AAE decoder).

Same algorithm as kernel.py (exact-sampling via host-precomputed gumbel,
one-hot embedding matmul, batch data-parallel over 8 cores x 4 passes of 512
rows), but the recurrent gate matmuls run as fp16 DUAL-PLANE products:
  W = Whi + Wlo/2048,  h = hhi + hlo/2048   (fp16 planes, lo pre-scaled)
  W@h ~= [Whi@hhi] + ([Whi@hlo + Wlo@hhi]) / 2048      (lo*lo term dropped)
Three fp16 matmuls (1 cyc/row) replace one fp32 matmul (4 cyc/row) at a
~2^-22 relative residual - fp32-grade. The logits matmul stays genuine fp32
(h1 kept in f32 for it), so the sampled scores keep full precision.
All weight planes are prepared host-side in the transposed SBUF layout.
"""
import sys

sys.path.insert(0, "/opt/trn_rl_repo")

import numpy as np

B, V, DEMB, H, LAT = 16384, 64, 128, 512, 128
PAD, BOS, EOS = 0, 1, 2
T = 100
NCORES = 8
BLOC = B // NCORES       # 2048 rows per core
QROWS = 512              # rows per pass
NPASS = BLOC // QROWS    # 4
P = 128
NQ = QROWS // P          # 4 m-subtiles per pass
KH = H // P              # 4 k-tiles of hidden
LSCALE = 2048.0          # lo-plane pre-scale (2^11)
F16_MIN_NORMAL = 6.104e-05


def legalize_waits(nc, max_waits=1):
    """walrus refuses >1 semaphore wait on many instruction types; split the
    excess onto chained same-engine NOPs placed right before the instruction
    (sequential execution on one engine makes this semantically identical)."""
    import concourse.mybir as mybir

    n_split = 0
    for fn in nc.m.functions:
        for bb in fn.blocks:
            insts = bb.instructions
            out = []
            for inst in insts:
                si = getattr(inst, "sync_info", None)
                waits = list(si.on_wait) if si is not None and si.on_wait else []
                if len(waits) > max_waits:
                    n_split += 1
                    head, tail = waits[:-max_waits], waits[-max_waits:]
                    for i in range(0, len(head), max_waits):
                        chunk = head[i:i + max_waits]
                        out.append(mybir.InstNoOp(
                            name=f"{inst.name}-waitsplit{i}",
                            engine=inst.engine,
                            ins=[],
                            outs=[],
                            sync_info=mybir.SyncInfo(on_wait=chunk, on_update=[]),
                        ))
                    si.on_wait = tail
                out.append(inst)
            if len(out) != len(insts):
                insts[:] = out
    return n_split


def build_nc(nsteps=T, reps=1):
    import contextlib
    import concourse.bass as bass
    import concourse.mybir as mybir
    from concourse.bass import ds
    from concourse.tile import TileContext

    f32 = mybir.dt.float32
    f16 = mybir.dt.float16
    i32 = mybir.dt.int32
    AF = mybir.ActivationFunctionType
    OP = mybir.AluOpType
    AX = mybir.AxisListType

    nc = bass.Bass()

    z_p = nc.declare_dram_parameter("z", [BLOC, LAT], f32, isOutput=False)
    # fp16 weight planes, already in [p, k, j] transposed layout
    w0h_p = nc.declare_dram_parameter("w0h", [P, KH, 4 * H], f16, isOutput=False)
    w0l_p = nc.declare_dram_parameter("w0l", [P, KH, 4 * H], f16, isOutput=False)
    w1ih_h_p = nc.declare_dram_parameter("w1ihh", [P, KH, 4 * H], f16, isOutput=False)
    w1ih_l_p = nc.declare_dram_parameter("w1ihl", [P, KH, 4 * H], f16, isOutput=False)
    w1hh_h_p = nc.declare_dram_parameter("w1hhh", [P, KH, 4 * H], f16, isOutput=False)
    w1hh_l_p = nc.declare_dram_parameter("w1hhl", [P, KH, 4 * H], f16, isOutput=False)
    xw0h_p = nc.declare_dram_parameter("xw0h", [V, 4 * H], f16, isOutput=False)
    xw0l_p = nc.declare_dram_parameter("xw0l", [V, 4 * H], f16, isOutput=False)
    g_p = nc.declare_dram_parameter("G", [T, BLOC, V], f32, isOutput=False)
    # consts: [0:128 ident | 128:192 negio | 192:208 b0T | 208:224 b1T |
    #          224:228 bl2hT | 228:484 WoutT (k*64+v) | 484:996 Wl2hT (k*128+l)]
    c_p = nc.declare_dram_parameter("consts", [P, 996], f32, isOutput=False)
    bos_p = nc.declare_dram_parameter("bosrow", [1, QROWS], f16, isOutput=False)
    out_p = nc.declare_dram_parameter("out", [BLOC, nsteps], i32, isOutput=True)

    with TileContext(nc) as tc:
        with (
            tc.tile_pool(name="wpool", bufs=1) as wpool,
            tc.tile_pool(name="spool", bufs=1) as spool,
            tc.tile_pool(name="wk", bufs=2) as wk,
            tc.tile_pool(name="gpsum", bufs=3, space="PSUM") as gpsum,
            tc.tile_pool(name="lpsum", bufs=1, space="PSUM") as lpsum,
            tc.tile_pool(name="tpsum", bufs=1, space="PSUM") as tpsum,
        ):
            cw = wpool.tile([P, 996], f32, tag="cw")
            nc.sync.dma_start(cw[:], c_p[:])
            ident = cw[:, 0:128]
            negio = cw[:, 128:192]
            b0T = cw[:, 192:208]
            b1T = cw[:, 208:224]
            bl2hT = cw[:, 224:228]

            def WoutT(k):
                return cw[:, 228 + k * V:228 + (k + 1) * V]

            def Wl2hT(k):
                return cw[:, 484 + k * P:484 + (k + 1) * P]

            w0h = wpool.tile([P, KH, 4 * H], f16, tag="w0h")
            w0l = wpool.tile([P, KH, 4 * H], f16, tag="w0l")
            w1ihh = wpool.tile([P, KH, 4 * H], f16, tag="w1ihh")
            w1ihl = wpool.tile([P, KH, 4 * H], f16, tag="w1ihl")
            w1hhh = wpool.tile([P, KH, 4 * H], f16, tag="w1hhh")
            w1hhl = wpool.tile([P, KH, 4 * H], f16, tag="w1hhl")
            xw0h = wpool.tile([V, 4 * H], f16, tag="xw0h")
            xw0l = wpool.tile([V, 4 * H], f16, tag="xw0l")
            for t_, p_ in ((w0h, w0h_p), (w0l, w0l_p), (w1ihh, w1ih_h_p),
                           (w1ihl, w1ih_l_p), (w1hhh, w1hh_h_p),
                           (w1hhl, w1hh_l_p), (xw0h, xw0h_p), (xw0l, xw0l_p)):
                nc.sync.dma_start(t_[:], p_[:])

            # ---------------- persistent state ----------------
            def st(name, dtype=f16):
                return spool.tile([P, KH, QROWS], dtype, tag=name, name=name)

            hA0h, hA0l = st("hA0h"), st("hA0l")
            hB0h, hB0l = st("hB0h"), st("hB0l")
            hA1h, hA1l = st("hA1h"), st("hA1l")
            hB1h, hB1l = st("hB1h"), st("hB1l")
            c0 = st("c0", f32)
            c1 = st("c1", f32)
            h1f32 = st("h1f32", f32)
            ohP = spool.tile([V, 2, QROWS], f16, tag="ohP")
            ohA = ohP[:, 0, :]
            ohB = ohP[:, 1, :]
            smx = spool.tile([P, 1548], f32, tag="smx")
            outb = spool.tile([P, NQ, nsteps // 2, 2], i32, tag="outb")

            g_view = g_p[:].rearrange("(s u) (q m p) v -> q u p s m v",
                                      u=2, q=NPASS, p=P)

            def gates_layer(biasT, hi_terms, cross_terms, oh_pair, cstate,
                            store_h):
                """hi_terms: [(Wplane, hplane)] -> ps1; cross_terms -> ps2;
                oh_pair: (xh, xl, ohr) or None. comb = ps1 + ps2/LSCALE."""
                for q in range(KH):
                    ag = [None] * 4
                    for g in range(4):
                        j = g * KH + q
                        jsl = slice(j * P, (j + 1) * P)
                        ps1 = gpsum.tile([P, QROWS], f32, tag="gp1", name="ps1")
                        ps2 = gpsum.tile([P, QROWS], f32, tag="gp2", name="ps2")
                        n1 = len(hi_terms) * KH + (1 if oh_pair else 0)
                        mi = 0
                        if oh_pair:
                            mi += 1
                            nc.tensor.matmul(ps1[:], oh_pair[0][:, jsl],
                                             oh_pair[2], start=True,
                                             stop=(mi == n1))
                        for (W, hp) in hi_terms:
                            for k in range(KH):
                                mi += 1
                                nc.tensor.matmul(
                                    ps1[:], W[:, k, jsl], hp[:, k, :],
                                    start=(mi == 1), stop=(mi == n1))
                        n2 = len(cross_terms) * KH + (1 if oh_pair else 0)
                        mi = 0
                        if oh_pair:
                            mi += 1
                            nc.tensor.matmul(ps2[:], oh_pair[1][:, jsl],
                                             oh_pair[2], start=True,
                                             stop=(mi == n2))
                        for (W, hp) in cross_terms:
                            for k in range(KH):
                                mi += 1
                                nc.tensor.matmul(
                                    ps2[:], W[:, k, jsl], hp[:, k, :],
                                    start=(mi == 1), stop=(mi == n2))
                        comb = wk.tile([P, QROWS], f32, tag="comb")
                        nc.vector.tensor_scalar(comb[:], ps2[:], 1.0 / LSCALE,
                                                None, OP.mult)
                        nc.vector.tensor_tensor(comb[:], comb[:], ps1[:],
                                                OP.add)
                        agt = wk.tile([P, QROWS], f32, tag=f"ag{g}", bufs=1,
                                      name=f"ag{g}")
                        fn = AF.Tanh if g == 2 else AF.Sigmoid
                        nc.scalar.activation(agt[:], comb[:], fn,
                                             bias=biasT[:, j:j + 1])
                        ag[g] = agt
                    t1 = wk.tile([P, QROWS], f32, tag="t1")
                    nc.vector.tensor_tensor(t1[:], ag[1][:], cstate[:, q, :],
                                            OP.mult)
                    t2 = wk.tile([P, QROWS], f32, tag="t2")
                    nc.vector.tensor_tensor(t2[:], ag[0][:], ag[2][:], OP.mult)
                    nc.vector.tensor_tensor(cstate[:, q, :], t1[:], t2[:],
                                            OP.add)
                    tch = wk.tile([P, QROWS], f32, tag="t1", name="tch")
                    nc.scalar.activation(tch[:], cstate[:, q, :], AF.Tanh)
                    store_h(q, ag[3], tch)

            def split_h(q, hnew_f32, hi_dst, lo_dst):
                """hi = f16(h); lo = f16((h - hi) * 2048)."""
                nc.vector.tensor_copy(hi_dst[:, q, :], hnew_f32)
                d = wk.tile([P, QROWS], f32, tag="t2", name="dsplit")
                nc.vector.tensor_tensor(d[:], hnew_f32, hi_dst[:, q, :],
                                        OP.subtract)
                nc.vector.tensor_scalar(lo_dst[:, q, :], d[:], LSCALE, None,
                                        OP.mult)

            def step(s, sub, h0rh, h0rl, h0wh, h0wl, h1rh, h1rl, h1wh, h1wl,
                     ohr, ohw, gum):
                # ---- layer 0 ----
                def store0(q, ago, tch):
                    hn = wk.tile([P, QROWS], f32, tag="t2", name="hnew0")
                    nc.vector.tensor_tensor(hn[:], ago[:], tch[:], OP.mult)
                    split_h(q, hn[:], h0wh, h0wl)

                gates_layer(b0T, [(w0h, h0rh)],
                            [(w0l, h0rh), (w0h, h0rl)],
                            (xw0h, xw0l, ohr[:]), c0, store0)

                # ---- layer 1 ----
                def store1(q, ago, tch):
                    nc.vector.tensor_tensor(h1f32[:, q, :], ago[:], tch[:],
                                            OP.mult)
                    split_h(q, h1f32[:, q, :], h1wh, h1wl)

                gates_layer(b1T, [(w1ihh, h0wh), (w1hhh, h1rh)],
                            [(w1ihl, h0wh), (w1ihh, h0wl),
                             (w1hhl, h1rh), (w1hhh, h1rl)],
                            None, c1, store1)

                # ---- logits + sampling (full fp32) ----
                for m in range(NQ):
                    pl = lpsum.tile([P, V], f32, tag="lp")
                    for k in range(KH):
                        nc.tensor.matmul(pl[:], h1f32[:, k, m * P:(m + 1) * P],
                                         WoutT(k),
                                         start=(k == 0), stop=(k == KH - 1))
                    o = m * 384
                    sc = smx[:, o + 0:o + 64]
                    eq = smx[:, o + 64:o + 128]
                    msk = smx[:, o + 128:o + 192]
                    oh = smx[:, o + 192:o + 256]
                    ohm = smx[:, o + 256:o + 320]
                    mx = smx[:, o + 320:o + 321]
                    rr = smx[:, o + 321:o + 322]
                    idx = smx[:, o + 322:o + 323]
                    eosn = smx[:, o + 323:o + 324]
                    ie = smx[:, o + 324:o + 325]
                    alive = smx[:, o + 325:o + 326]
                    iend = smx[:, 1536 + m:1537 + m]
                    nc.vector.tensor_tensor(sc, pl[:], gum[:, sub, m, :],
                                            OP.add)
                    nc.vector.reduce_max(mx, sc, axis=AX.X)
                    nc.vector.tensor_scalar(eq, sc, mx, None, OP.is_equal)
                    nc.vector.tensor_tensor(msk, eq, negio, OP.mult)
                    nc.vector.reduce_max(rr, msk, axis=AX.X)
                    nc.vector.tensor_scalar(idx, rr, -1.0, float(V),
                                            OP.mult, OP.add)
                    nc.vector.tensor_scalar(eosn, idx, float(EOS), None,
                                            OP.is_equal)
                    nc.vector.tensor_tensor(ie, iend, eosn, OP.max)
                    nc.vector.tensor_copy(iend, ie)
                    nc.vector.tensor_scalar(alive, ie, -1.0, 1.0,
                                            OP.mult, OP.add)
                    tok = smx[:, 1540 + m:1541 + m]
                    nc.vector.tensor_tensor(tok, idx, alive, OP.mult)
                    nc.vector.tensor_scalar(oh, negio, rr, None, OP.is_equal)
                    nc.vector.tensor_scalar(ohm, oh, alive, None, OP.mult)
                    tp = tpsum.tile([P, P], f32, tag="tp")
                    nc.tensor.transpose(tp[:V, :], ohm, ident)
                    nc.scalar.copy(ohw[:, m * P:(m + 1) * P], tp[:V, :])
                nc.vector.tensor_copy(outb[:, :, ds(s, 1), sub:sub + 1],
                                      smx[:, 1540:1544])

            rep_ctx = tc.For_i(0, reps) if reps > 1 else contextlib.nullcontext()
            with rep_ctx:
              for q_pass in range(NPASS):
                for s_ in (hA0h, hA0l, hB0h, hB0l, hA1h, hA1l, hB1h, hB1l,
                           h1f32):
                    nc.vector.memset(s_[:], 0.0)
                nc.vector.memset(smx[:, 1536:1540], 0.0)
                nc.vector.memset(ohP[:], 0.0)
                nc.sync.dma_start(ohA[BOS:BOS + 1, :], bos_p[:])
                zT = wk.tile([P, QROWS], f32, tag="ag0", bufs=1, name="zT")
                for m in range(NQ):
                    znat = wk.tile([P, LAT], f32, tag="ag1", bufs=1,
                                   name="znat")
                    r0 = q_pass * QROWS + m * P
                    nc.sync.dma_start(znat[:], z_p[r0:r0 + P, :])
                    tp = tpsum.tile([P, P], f32, tag="tp")
                    nc.tensor.transpose(tp[:], znat[:], ident)
                    nc.vector.tensor_copy(zT[:, m * P:(m + 1) * P], tp[:])
                for k in range(KH):
                    cp = gpsum.tile([P, QROWS], f32, tag="gp1", name="cinit")
                    nc.tensor.matmul(cp[:], Wl2hT(k), zT[:],
                                     start=True, stop=True)
                    nc.vector.tensor_scalar(c0[:, k, :], cp[:],
                                            bl2hT[:, k:k + 1], None, OP.add)
                    nc.vector.tensor_copy(c1[:, k, :], c0[:, k, :])

                with tc.For_i(0, nsteps // 2,
                              hint_engines=(mybir.EngineType.PE,
                                            mybir.EngineType.DVE)) as s:
                    gum = wk.tile([P, 2, NQ, V], f32, tag="gum", bufs=1,
                                  name="gum")
                    for u in range(2):
                        nc.sync.dma_start(gum[:, u:u + 1, :, :],
                                          g_view[q_pass, u, :, ds(s, 1), :, :])
                    step(s, 0, hA0h, hA0l, hB0h, hB0l, hA1h, hA1l,
                         hB1h, hB1l, ohA, ohB, gum)
                    step(s, 1, hB0h, hB0l, hA0h, hA0l, hB1h, hB1l,
                         hA1h, hA1l, ohB, ohA, gum)

                ov = out_p[q_pass * QROWS:(q_pass + 1) * QROWS, :]
                nc.sync.dma_start(
                    ov.rearrange("(m p) (s u) -> p m s u", p=P, u=2), outb[:])

    legalize_waits(nc)
    return nc


def _split16(W):
    """fp16 dual planes with subnormal-flushed hi."""
    hi = W.astype(np.float16)
    hi[np.abs(hi.astype(np.float32)) < F16_MIN_NORMAL] = 0
    lo = ((W - hi.astype(np.float32)) * LSCALE).astype(np.float16)
    return hi, lo


def _tlayout(W):
    """[4H, K] weight -> [p, k, j] transposed plane layout ([128, KH, 4H])."""
    K = W.shape[1]
    return np.ascontiguousarray(
        W.T.reshape(K // P, P, 4 * H).transpose(1, 0, 2))


def _host_consts(b_ih0, b_hh0, b_ih1, b_hh1, b_l2h, W_out, W_l2h):
    consts = np.zeros((P, 996), np.float32)
    consts[:, 0:128] = np.eye(P, dtype=np.float32)
    consts[:, 128:192] = (V - np.arange(V, dtype=np.float32))[None, :]
    consts[:, 192:208] = (b_ih0 + b_hh0).astype(np.float32).reshape(16, P).T
    consts[:, 208:224] = (b_ih1 + b_hh1).astype(np.float32).reshape(16, P).T
    consts[:, 224:228] = b_l2h.astype(np.float32).reshape(KH, P).T
    # WoutT[p, k, v] = W_out[v, k*128+p]
    wt = W_out.astype(np.float32).T.reshape(KH, P, V).transpose(1, 0, 2)
    consts[:, 228:484] = wt.reshape(P, KH * V)
    # Wl2hT[p(l), k, m] = W_l2h[k*128+m, p(l)]
    wl = W_l2h.astype(np.float32).reshape(KH, P, LAT).transpose(2, 0, 1)
    consts[:, 484:996] = wl.reshape(P, KH * P)
    return consts


def _host_gumbel(b_out):
    """Exactly the noise jax.random.categorical(keys[t], logits) adds."""
    import jax

    cpu = jax.devices("cpu")[0]
    with jax.default_device(cpu):
        keys = jax.random.split(jax.random.key(42), T)

        def one(k):
            return jax.random.gumbel(k, (B, V), "float32")

        one_j = jax.jit(one)
        # NOTE: vmap over keys generates different bits than per-key calls;
        # the reference samples per key, so loop.
        g = np.stack([np.asarray(one_j(keys[t])) for t in range(T)])
    g = g + b_out.astype(np.float32)[None, None, :]
    return g


def prepare_in_maps(inputs):
    """Build the per-core SPMD input maps from the full problem inputs."""
    f32 = np.float32
    E = np.asarray(inputs["E"], f32)
    W_ih0 = np.asarray(inputs["W_ih0"], f32)
    consts = _host_consts(np.asarray(inputs["b_ih0"]), np.asarray(inputs["b_hh0"]),
                          np.asarray(inputs["b_ih1"]), np.asarray(inputs["b_hh1"]),
                          np.asarray(inputs["b_l2h"]),
                          np.asarray(inputs["W_out"], f32),
                          np.asarray(inputs["W_l2h"], f32))
    G = _host_gumbel(np.asarray(inputs["b_out"]))

    XW0 = (E @ W_ih0.T).astype(f32)               # [64, 2048]
    xw0h, xw0l = _split16(XW0)
    w0h, w0l = _split16(_tlayout(np.asarray(inputs["W_hh0"], f32)))
    w1ihh, w1ihl = _split16(_tlayout(np.asarray(inputs["W_ih1"], f32)))
    w1hhh, w1hhl = _split16(_tlayout(np.asarray(inputs["W_hh1"], f32)))

    shared = {
        "w0h": w0h, "w0l": w0l, "w1ihh": w1ihh, "w1ihl": w1ihl,
        "w1hhh": w1hhh, "w1hhl": w1hhl,
        "xw0h": np.ascontiguousarray(xw0h),
        "xw0l": np.ascontiguousarray(xw0l),
        "consts": consts,
        "bosrow": np.ones((1, QROWS), np.float16),
    }
    z = np.asarray(inputs["z"], f32)
    in_maps = []
    for i in range(NCORES):
        rows = slice(i * BLOC, (i + 1) * BLOC)
        m = dict(shared)
        m["z"] = np.ascontiguousarray(z[rows])
        m["G"] = np.ascontiguousarray(G[:, rows, :])
        in_maps.append(m)
    return in_maps


def kernel(z, E, W_l2h, b_l2h, W_ih0, W_hh0, b_ih0, b_hh0,
           W_ih1, W_hh1, b_ih1, b_hh1, W_out, b_out, max_len):
    from concourse.bass_utils import run_bass_kernel_spmd

    assert int(max_len) == T
    in_maps = prepare_in_maps(dict(
        z=z, E=E, W_l2h=W_l2h, b_l2h=b_l2h, W_ih0=W_ih0, W_hh0=W_hh0,
        b_ih0=b_ih0, b_hh0=b_hh0, W_ih1=W_ih1, W_hh1=W_hh1, b_ih1=b_ih1,
        b_hh1=b_hh1, W_out=W_out, b_out=b_out))
    nc = build_nc()
    res = run_bass_kernel_spmd(nc, in_maps, list(range(NCORES)))
    out = np.concatenate([res.results[i]["out"] for i in range(NCORES)], axis=0)
    return out.astype(np.int32)
